# revision 7
# baseline (speedup 1.0000x reference)
"""Multi-head GAT layer for Trainium2 — 8 heads sharded across 8 NeuronCores.

Per head h (N=4096 nodes, F=64 features):
    ltg   = graph @ W[h]                          [N, F]
    s     = ltg @ a_src,  d = ltg @ a_dst         [N]
    E     = leaky_relu(s[:, None] + d[None, :], 0.2)
    Alpha = softmax(E, axis=-1)
    out   = Alpha @ ltg

Algebraic trick (exact): with z = s_i + d_j and M_ij = [z >= 0],

    exp(leaky_relu(z)) = M_ij e^{s_i} e^{d_j} + (1-M_ij) e^{.2 s_i} e^{.2 d_j}

so the N x N softmax reduces to one mask-materialization pass plus masked
matmuls on the PE against R = [v .* ltg | v2 .* ltg | v | v2] (130 bf16
columns per j-block), with the (1-M) path recovered as total-minus-masked.

Mask generation is split across THREE engines with consistent algebra:
  - DVE / GpSimd emit m = 2*[z >= 0]      in {0, 2}   (tensor_scalar is_ge, x2)
  - Act emits        m = sign(z)          in {-1, +1} (Sign lives in the same
    activation table as Exp — no 1.3us table reload)
For {0,2} blocks the matmul contributes 2*A_b; for sign blocks 2*A_b - T_b
(T_b = column totals of R_b).  Summed over j-blocks:
    P = 2*A_total - Tact,   Tact := sum of T_b over sign-assigned blocks,
so  A_total = (P + Tact)/2.  The +Tact is folded into the PSUM->SBUF
snapshot (tensor_tensor add) and the global 1/2 cancels in num/den.

Other changes vs the v1 kernel: bf16 projection/setup matmuls (f32 matmul
runs at 4 cycles/row), batched setup copies, totals accumulated on the fly,
mask inputs read s via a partition-broadcast view of the s-row (no [P,N]
s_rep materialization), PSUM accumulator double-buffered across supersteps,
and masks prefetched one superstep ahead.

Heads are fully independent: core h computes head h; no collectives.
"""

import os
from contextlib import ExitStack

import numpy as np

N, F_IN, F, H = 4096, 64, 64, 8
P = 128
NB = N // P           # 32 node blocks
ISUP = 4              # i-blocks per PSUM super-block (4 banks of accumulators)
NSUP = NB // ISUP     # 8 super iterations
RC = 130              # R columns per j-block: [v.ltg | v2.ltg | v | v2]

# j-block -> mask engine split (counts; blocks are assigned contiguously:
# first DVE, then GpSimd, then Act/sign)
NB_DVE = int(os.environ.get("GAT_NB_DVE", "16"))
NB_GP = int(os.environ.get("GAT_NB_GP", "9"))
NB_ACT = NB - NB_DVE - NB_GP
_CACHE = {}


def _build():
    import concourse.bass as bass  # noqa: F401
    import concourse.mybir as mybir
    import concourse.tile as tile
    from concourse import bacc

    dt = mybir.dt
    f32 = dt.float32
    bf16 = dt.bfloat16
    Alu = mybir.AluOpType
    Act = mybir.ActivationFunctionType

    gp_first = NB_DVE
    act_first = NB_DVE + NB_GP

    def mask_engine(b):
        if b < gp_first:
            return "dve"
        if b < act_first:
            return "gp"
        return "act"

    nc = bacc.Bacc("TRN2", debug=False, num_devices=H)
    graph_d = nc.dram_tensor("graph", [N, F_IN], f32, kind="ExternalInput").ap()
    w_d = nc.dram_tensor("w", [F_IN, F], f32, kind="ExternalInput").ap()
    a_d = nc.dram_tensor("a", [2, F], f32, kind="ExternalInput").ap()
    out_d = nc.dram_tensor("out", [N, F], f32, kind="ExternalOutput").ap()

    ident_d = nc.inline_tensor(np.eye(P, dtype=np.float32), name="ident")

    with tile.TileContext(nc) as tc, ExitStack() as ctx:
        persist = ctx.enter_context(tc.tile_pool(name="persist", bufs=1))
        gp = ctx.enter_context(tc.tile_pool(name="gp", bufs=8))
        mp = ctx.enter_context(tc.tile_pool(name="mask", bufs=2))
        ep = ctx.enter_context(tc.tile_pool(name="ep", bufs=6))
        ssb = ctx.enter_context(tc.tile_pool(name="ssb", bufs=2))

        identity = persist.tile([P, P], f32)
        nc.sync.dma_start(identity[:], ident_d.ap())
        ones_col_bf = persist.tile([P, 1], bf16)
        nc.vector.memset(ones_col_bf[:], 1.0)
        ones_row = persist.tile([1, P], f32)
        nc.vector.memset(ones_row[:], 1.0)
        ones_row_bf = persist.tile([1, P], bf16)
        nc.vector.memset(ones_row_bf[:], 1.0)

        wssd_f = persist.tile([F_IN, F + 2], f32)   # staging: [W | w_s | w_d]
        nc.sync.dma_start(wssd_f[:, 0:F], w_d[:])
        a2_sb = persist.tile([F, 2], f32)
        nc.sync.dma_start(a2_sb[:], a_d.rearrange("t k -> k t"))
        wssd = persist.tile([F_IN, F + 2], bf16)    # bf16 working copy

        gT = persist.tile([F_IN, N], bf16)           # graph^T (bf16)
        ltgsd = persist.tile([P, 66 * NB], f32)      # per b: ltg (64) | s | d
        negd = persist.tile([P, NB], f32)            # -d columns (is_ge bias)
        uv1 = persist.tile([P, 2 * NB], f32)         # exp(s), exp(d)
        uv2 = persist.tile([P, 2 * NB], f32)         # exp(.2 s), exp(.2 d)
        sdrow = persist.tile([2, N], bf16)           # s, d rows
        s_rep = persist.tile([P, N], bf16)           # s broadcast down parts
        r_all = persist.tile([P, RC * NB], bf16)     # [R1|R2|v|v2] per b
        t2rep = persist.tile([P, 196], f32)          # [T2(66) | Tact(130)] bcast
        c2rep = persist.tile([P, 66], f32)           # 2*T2 bcast
        eps_all = persist.tile([P, RC * ISUP * 2], f32)  # snapshots, ping-pong

        ltgsd_v = ltgsd.rearrange("p (b c) -> p b c", c=66)
        r_v = r_all.rearrange("p (b c) -> p b c", c=RC)
        uv1_v = uv1.rearrange("p (b c) -> p b c", c=2)
        uv2_v = uv2.rearrange("p (b c) -> p b c", c=2)

        mask_tiles = {}

        def emit_mask(sup, b):
            i0 = sup * ISUP * P
            srep = s_rep[:, i0:i0 + ISUP * P]
            mt = mp.tile([P, ISUP * P], bf16, tag=f"m{b}", name=f"mask{b}")
            eng = mask_engine(b)
            if eng == "act":
                # sign(s_i + d_j) in {-1, +1}; Sign shares the Exp act table
                nc.scalar.sign(mt[:], srep,
                               bias=ltgsd[:, 66 * b + 65:66 * b + 66])
            elif eng == "gp":
                nc.gpsimd.tensor_scalar(mt[:], srep, negd[:, b:b + 1], 2.0,
                                        op0=Alu.is_ge, op1=Alu.mult)
            else:
                nc.vector.tensor_scalar(mt[:], srep, negd[:, b:b + 1], 2.0,
                                        op0=Alu.is_ge, op1=Alu.mult)
            mask_tiles[(sup, b)] = mt

        # ---- setup: projection, s/d, R tables, totals ----
        with ExitStack() as sctx:
            sps = sctx.enter_context(
                tc.tile_pool(name="sps", bufs=2, space="PSUM"))
            sb2p = sctx.enter_context(
                tc.tile_pool(name="sb2", bufs=2, space="PSUM"))
            t2p = sctx.enter_context(
                tc.tile_pool(name="t2p", bufs=1, space="PSUM"))

            # W^T, then [w_s | w_d] = (W^T).T @ a2 ; all-f32, one-off
            wT_ps = sps.tile([F, F_IN], f32, tag="tp")
            nc.tensor.transpose(wT_ps[:], wssd_f[:, 0:F],
                                identity[0:F_IN, 0:F_IN])
            wT_sb = ssb.tile([F, F_IN], f32)
            nc.vector.tensor_copy(wT_sb[:], wT_ps[:])
            wsd_ps = sps.tile([F_IN, 2], f32, tag="pj")
            nc.tensor.matmul(wsd_ps[:], wT_sb[:], a2_sb[:])
            nc.vector.tensor_copy(wssd_f[:, F:F + 2], wsd_ps[:])
            nc.vector.tensor_copy(wssd[:], wssd_f[:])  # cast to bf16

            t2all_ps = t2p.tile([1, 196], f32, name="t2all")
            n_act_t2 = NB - act_first

            def do_group(g):
                """s/d tables + R blocks + totals for blocks 4g..4g+3."""
                sd_src = ltgsd_v[:, 4 * g:4 * g + 4, F:F + 2]
                d_src = ltgsd_v[:, 4 * g:4 * g + 4, F + 1]
                nc.vector.tensor_scalar(negd[:, 4 * g:4 * g + 4], d_src,
                                        -1.0, None, op0=Alu.mult)
                dst = slice(8 * g, 8 * g + 8)
                nc.scalar.activation(uv1[:, dst], sd_src, Act.Exp)
                nc.scalar.activation(uv2[:, dst], sd_src, Act.Exp, scale=0.2)
                for bb in range(4 * g, 4 * g + 4):
                    ltg_b = ltgsd[:, 66 * bb:66 * bb + F]
                    r0 = RC * bb
                    nc.vector.tensor_scalar(
                        r_all[:, r0:r0 + F], ltg_b,
                        uv1[:, 2 * bb + 1:2 * bb + 2], None, op0=Alu.mult)
                    nc.gpsimd.tensor_scalar(
                        r_all[:, r0 + F:r0 + 2 * F], ltg_b,
                        uv2[:, 2 * bb + 1:2 * bb + 2], None, op0=Alu.mult)
                bsl = slice(4 * g, 4 * g + 4)
                nc.vector.tensor_copy(r_v[:, bsl, 128], uv1_v[:, bsl, 1])
                nc.vector.tensor_copy(r_v[:, bsl, 129], uv2_v[:, bsl, 1])
                # totals: T2 (cols 0:66) over all blocks; Tact (cols 66:196)
                # over sign-assigned blocks.  One open accumulation region
                # pair in a single PSUM bank; per-byte zero-on-first-write.
                for bb in range(4 * g, 4 * g + 4):
                    r0 = RC * bb
                    nc.tensor.matmul(t2all_ps[0:1, 0:66], ones_col_bf[:],
                                     r_all[:, r0 + F:r0 + RC],
                                     start=(bb == 0), stop=(bb == NB - 1),
                                     skip_group_check=True)
                    if bb >= act_first:
                        nc.tensor.matmul(t2all_ps[0:1, 66:196],
                                         ones_col_bf[:],
                                         r_all[:, r0:r0 + RC],
                                         start=False, stop=(bb == NB - 1),
                                         skip_group_check=True)

            for g in range(NSUP):
                tp_ps = sps.tile([F_IN, ISUP * P], f32, tag="tp")
                for k in range(ISUP):
                    b = 4 * g + k
                    g_sb = gp.tile([P, F_IN], f32)
                    nc.sync.dma_start(g_sb[:], graph_d[b * P:(b + 1) * P, :])
                    nc.tensor.transpose(tp_ps[:, k * P:(k + 1) * P], g_sb[:],
                                        identity[:])
                nc.vector.tensor_copy(gT[:, 512 * g:512 * (g + 1)], tp_ps[:])
                pj_ps = sps.tile([P, 66 * ISUP], f32, tag="pj")
                for k in range(ISUP):
                    b = 4 * g + k
                    nc.tensor.matmul(pj_ps[:, 66 * k:66 * (k + 1)],
                                     gT[:, b * P:(b + 1) * P], wssd[:])
                nc.vector.tensor_copy(ltgsd[:, 264 * g:264 * (g + 1)],
                                      pj_ps[:])
                sr_ps = sb2p.tile([2, 512], f32, tag="sr")
                nc.tensor.matmul(sr_ps[:], wssd[:, F:F + 2],
                                 gT[:, 512 * g:512 * (g + 1)])
                nc.scalar.copy(sdrow[:, 512 * g:512 * (g + 1)], sr_ps[:])
                bc_ps = sb2p.tile([P, 512], f32, tag="sr", name="bc_ps")
                nc.tensor.matmul(bc_ps[:], ones_row_bf[:],
                                 sdrow[0:1, 512 * g:512 * (g + 1)])
                nc.scalar.copy(s_rep[:, 512 * g:512 * (g + 1)], bc_ps[:])
                do_group(g)
                for bb in range(4 * g, 4 * g + 4):
                    emit_mask(0, bb)

            # broadcast totals down partitions: [1,196] -> [128,196]
            t2all_sb = ssb.tile([1, 196], f32, tag="t2sb")
            nc.vector.tensor_copy(t2all_sb[:], t2all_ps[:])
            t2rep_ps = sps.tile([P, 196], f32, tag="tp")
            nc.tensor.matmul(t2rep_ps[:], ones_row[:], t2all_sb[:])
            nc.scalar.copy(t2rep[:], t2rep_ps[:])
            nc.vector.tensor_scalar(c2rep[:], t2rep[:, 0:66], 2.0, None,
                                    op0=Alu.mult)

        # ---- main masked-matmul loop ----
        accp = ctx.enter_context(
            tc.tile_pool(name="acc", bufs=2, space="PSUM"))

        tact_rep = t2rep[:, 66:196]  # [P, 130] view, aligned with acc cols

        def late_phase(sup):
            """Epilogue on SBUF snapshots eps = P + Tact (global 1/2 cancels
            in num/den)."""
            i0 = sup * ISUP
            e_base = (sup % 2) * RC * ISUP
            es = eps_all[:, e_base:e_base + RC * ISUP]
            es_v = es.rearrange("p (t c) -> p t c", c=RC)
            u_v = uv1_v[:, i0:i0 + ISUP, 0]
            u2_v = uv2_v[:, i0:i0 + ISUP, 0]
            den1 = ep.tile([P, ISUP], f32, tag="den1", name="den1")
            nc.vector.tensor_tensor(den1[:], u_v, es_v[:, :, 128],
                                    op=Alu.mult)
            dd = ep.tile([P, ISUP], f32, tag="dd", name="dd")
            nc.vector.tensor_tensor(dd[:],
                                    c2rep[:, 65:66].to_broadcast([P, ISUP]),
                                    es_v[:, :, 129], op=Alu.subtract)
            den2 = ep.tile([P, ISUP], f32, tag="den2", name="den2")
            nc.vector.tensor_tensor(den2[:], dd[:], u2_v, op=Alu.mult)
            den = ep.tile([P, ISUP], f32, tag="den", name="den")
            nc.vector.tensor_tensor(den[:], den2[:], den1[:], op=Alu.add)
            rden = ep.tile([P, ISUP], f32, tag="rden", name="rden")
            nc.vector.reciprocal(rden[:], den[:])
            for t in range(ISUP):
                i = i0 + t
                e0 = e_base + RC * t
                u_col = uv1[:, 2 * i:2 * i + 1]
                u2_col = uv2[:, 2 * i:2 * i + 1]
                n1 = ep.tile([P, F], f32, tag="n1", name="n1")
                nc.scalar.mul(n1[:], eps_all[:, e0:e0 + F], u_col)
                d2 = ep.tile([P, F], f32, tag="d2", name="d2")
                nc.vector.tensor_tensor(d2[:], c2rep[:, 0:F],
                                        eps_all[:, e0 + F:e0 + 2 * F],
                                        op=Alu.subtract)
                n2 = ep.tile([P, F], f32, tag="n2", name="n2")
                nc.scalar.mul(n2[:], d2[:], u2_col)
                num = ep.tile([P, F], f32, tag="num", name="num")
                nc.vector.tensor_tensor(num[:], n1[:], n2[:], op=Alu.add)
                ot = ep.tile([P, F], f32, tag="ot", name="ot")
                nc.scalar.mul(ot[:], num[:], rden[:, t:t + 1])
                nc.sync.dma_start(out_d[i * P:(i + 1) * P, :], ot[:])

        for sup in range(NSUP):
            if sup + 1 < NSUP:
                for b in range(NB):
                    emit_mask(sup + 1, b)
            mtiles = [mask_tiles.pop((sup, b)) for b in range(NB)]

            acc = accp.tile([P, 512 * ISUP], f32, tag="acc", name=f"acc{sup}")
            for b in range(NB):
                r0 = RC * b
                for t in range(ISUP):
                    nc.tensor.matmul(
                        acc[:, 512 * t:512 * t + RC],
                        mtiles[b][:, t * P:(t + 1) * P],
                        r_all[:, r0:r0 + RC],
                        start=(b == 0), stop=(b == NB - 1))
            # snapshot psum -> SBUF with the +Tact correction folded in
            e_base = (sup % 2) * RC * ISUP
            for t in range(ISUP):
                nc.vector.tensor_tensor(
                    eps_all[:, e_base + RC * t:e_base + RC * (t + 1)],
                    acc[:, 512 * t:512 * t + RC], tact_rep, op=Alu.add)
            if sup > 0:
                late_phase(sup - 1)
        late_phase(NSUP - 1)

    nc.compile()
    return nc


def _get_nc():
    if "nc" not in _CACHE:
        _CACHE["nc"] = _build()
    return _CACHE["nc"]


def kernel(graph, W, a):
    from concourse.bass_utils import run_bass_kernel_spmd

    graph = np.ascontiguousarray(np.asarray(graph, dtype=np.float32))
    W = np.asarray(W, dtype=np.float32)
    a = np.asarray(a, dtype=np.float32)

    nc = _get_nc()
    in_maps = [
        {
            "graph": graph,
            "w": np.ascontiguousarray(W[h]),
            "a": np.ascontiguousarray(a[h].reshape(2, F)),
        }
        for h in range(H)
    ]
    trace = bool(int(os.environ.get("GAT_TRACE", "0")))
    res = run_bass_kernel_spmd(nc, in_maps, core_ids=list(range(H)), trace=trace)
    _CACHE["last_result"] = res
    return np.stack([res.results[h]["out"] for h in range(H)], axis=0)


# revision 11
# speedup vs baseline: 5.2554x; 5.2554x over previous
"""Multi-head GAT layer for Trainium2 — 8 heads sharded across 8 NeuronCores.

Per head h (N=4096 nodes, F=64 features):
    ltg   = graph @ W[h]                          [N, F]
    s     = ltg @ a_src,  d = ltg @ a_dst         [N]
    E     = leaky_relu(s[:, None] + d[None, :], 0.2)
    Alpha = softmax(E, axis=-1)
    out   = Alpha @ ltg

Algebraic trick (exact): with z = s_i + d_j and M_ij = [z >= 0],

    exp(leaky_relu(z)) = M_ij e^{s_i} e^{d_j} + (1-M_ij) e^{.2 s_i} e^{.2 d_j}

so the N x N softmax reduces to one mask-materialization pass plus masked
matmuls on the PE against R = [v .* ltg | v2 .* ltg | v | v2] (130 bf16
columns per j-block), with the (1-M) path recovered as total-minus-masked.

Mask generation is split across DVE and the Activation engine with
consistent algebra:
  - DVE blocks emit m = [z >= 0] in {0, 1} (single-op tensor_scalar is_ge;
    two-op forms and GpSimd fall off the HW fast path by 10-30x).
  - Act blocks emit m = sign(z) in {-1, +1} (Sign shares the Exp activation
    table — no 1.3us table reload), and use HALVED R columns
    (vh = e^{d - ln2} etc.), so their matmul contribution is
    sign @ (r/2) = A_b - T_b/2.
Summing over j-blocks:  P = A_total - TactHalf, with TactHalf the column
totals of the halved R over sign-assigned blocks.  The +TactHalf correction
is folded into the PSUM->SBUF snapshot, after which eps == A_total exactly
and the plain complement algebra applies.  Sign blocks contribute their
UNHALVED totals to T2 via a twos-valued stationary vector.

vs the v1 kernel: bf16 projection/setup matmuls (f32 matmul runs at 4
cycles/row), batched setup copies, totals accumulated on the fly, PSUM
accumulator double-buffered across supersteps, masks prefetched one
superstep ahead, and the whole epilogue on DVE with broadcast multipliers.

Heads are fully independent: core h computes head h; no collectives.
"""

import math
import os
from contextlib import ExitStack

import numpy as np

N, F_IN, F, H = 4096, 64, 64, 8
P = 128
NB = N // P           # 32 node blocks
ISUP = 4              # i-blocks per PSUM super-block (4 banks of accumulators)
NSUP = NB // ISUP     # 8 super iterations
RC = 130              # R columns per j-block: [v.ltg | v2.ltg | v | v2]

# j-block -> mask engine split: blocks [0, NB-NB_ACT) on DVE (is_ge {0,1}),
# blocks [NB-NB_ACT, NB) on Act (sign {-1,+1} with halved R)
NB_ACT = int(os.environ.get("GAT_NB_ACT", "14"))
_CACHE = {}


def _build():
    import concourse.bass as bass  # noqa: F401
    import concourse.mybir as mybir
    import concourse.tile as tile
    from concourse import bacc

    dt = mybir.dt
    f32 = dt.float32
    bf16 = dt.bfloat16
    Alu = mybir.AluOpType
    Act = mybir.ActivationFunctionType

    act_first = NB - NB_ACT
    LN2 = float(math.log(2.0))

    nc = bacc.Bacc("TRN2", debug=False, num_devices=H)
    graph_d = nc.dram_tensor("graph", [N, F_IN], f32, kind="ExternalInput").ap()
    w_d = nc.dram_tensor("w", [F_IN, F], f32, kind="ExternalInput").ap()
    a_d = nc.dram_tensor("a", [2, F], f32, kind="ExternalInput").ap()
    out_d = nc.dram_tensor("out", [N, F], f32, kind="ExternalOutput").ap()

    ident_d = nc.inline_tensor(np.eye(P, dtype=np.float32), name="ident")

    with tile.TileContext(nc) as tc, ExitStack() as ctx:
        persist = ctx.enter_context(tc.tile_pool(name="persist", bufs=1))
        gpool = ctx.enter_context(tc.tile_pool(name="gp", bufs=8))
        mp = ctx.enter_context(tc.tile_pool(name="mask", bufs=2))
        ep = ctx.enter_context(tc.tile_pool(name="ep", bufs=6))
        ssb = ctx.enter_context(tc.tile_pool(name="ssb", bufs=2))

        identity = persist.tile([P, P], f32)
        nc.sync.dma_start(identity[:], ident_d.ap())
        ones_col_bf = persist.tile([P, 1], bf16)
        nc.vector.memset(ones_col_bf[:], 1.0)
        twos_col_bf = persist.tile([P, 1], bf16)
        nc.vector.memset(twos_col_bf[:], 2.0)
        ones_row = persist.tile([1, P], f32)
        nc.vector.memset(ones_row[:], 1.0)
        ones_row_bf = persist.tile([1, P], bf16)
        nc.vector.memset(ones_row_bf[:], 1.0)
        nln2_col = persist.tile([P, 1], f32)
        nc.vector.memset(nln2_col[:], -LN2)

        wssd_f = persist.tile([F_IN, F + 2], f32)   # staging: [W | w_s | w_d]
        nc.sync.dma_start(wssd_f[:, 0:F], w_d[:])
        a2_sb = persist.tile([F, 2], f32)
        nc.sync.dma_start(a2_sb[:], a_d.rearrange("t k -> k t"))
        wssd = persist.tile([F_IN, F + 2], bf16)    # bf16 working copy

        gT = persist.tile([F_IN, N], bf16)           # graph^T (bf16)
        ltgsd = persist.tile([P, 66 * NB], f32)      # per b: ltg (64) | s | d
        negd = persist.tile([P, NB], f32)            # -d columns (is_ge bias)
        uv1 = persist.tile([P, 2 * NB], f32)         # exp(s), exp(d)
        uv2 = persist.tile([P, 2 * NB], f32)         # exp(.2 s), exp(.2 d)
        uvh1 = persist.tile([P, 2 * NB], f32)        # exp(. - ln2) (act blocks)
        uvh2 = persist.tile([P, 2 * NB], f32)
        sdrow = persist.tile([2, N], bf16)           # s, d rows
        s_rep = persist.tile([P, N], bf16)           # s broadcast down parts
        r_all = persist.tile([P, RC * NB], bf16)     # [R1|R2|v|v2] per b
        t2rep = persist.tile([P, 196], f32)          # [T2(66)|TactHalf(130)]
        tact4 = persist.tile([P, RC * ISUP], f32)    # TactHalf repeated 4x
        eps_all = persist.tile([P, RC * ISUP * 2], f32)  # snapshots ping-pong

        ltgsd_v = ltgsd.rearrange("p (b c) -> p b c", c=66)
        r_v = r_all.rearrange("p (b c) -> p b c", c=RC)
        uv1_v = uv1.rearrange("p (b c) -> p b c", c=2)
        uv2_v = uv2.rearrange("p (b c) -> p b c", c=2)
        uvh1_v = uvh1.rearrange("p (b c) -> p b c", c=2)
        uvh2_v = uvh2.rearrange("p (b c) -> p b c", c=2)

        mask_tiles = {}

        def emit_mask(sup, b):
            i0 = sup * ISUP * P
            srep = s_rep[:, i0:i0 + ISUP * P]
            mt = mp.tile([P, ISUP * P], bf16, tag=f"m{b}", name=f"mask{b}")
            if b >= act_first:
                # sign(s_i + d_j) in {-1,+1}; Sign shares the Exp act table
                nc.scalar.sign(mt[:], srep,
                               bias=ltgsd[:, 66 * b + 65:66 * b + 66])
            else:
                nc.vector.tensor_scalar(mt[:], srep, negd[:, b:b + 1], None,
                                        op0=Alu.is_ge)
            mask_tiles[(sup, b)] = mt

        # ---- setup: projection, s/d, R tables, totals ----
        with ExitStack() as sctx:
            sps = sctx.enter_context(
                tc.tile_pool(name="sps", bufs=2, space="PSUM"))
            sb2p = sctx.enter_context(
                tc.tile_pool(name="sb2", bufs=2, space="PSUM"))
            t2p = sctx.enter_context(
                tc.tile_pool(name="t2p", bufs=1, space="PSUM"))

            # W^T, then [w_s | w_d] = (W^T).T @ a2 ; all-f32, one-off
            wT_ps = sps.tile([F, F_IN], f32, tag="tp")
            nc.tensor.transpose(wT_ps[:], wssd_f[:, 0:F],
                                identity[0:F_IN, 0:F_IN])
            wT_sb = ssb.tile([F, F_IN], f32)
            nc.vector.tensor_copy(wT_sb[:], wT_ps[:])
            wsd_ps = sps.tile([F_IN, 2], f32, tag="pj")
            nc.tensor.matmul(wsd_ps[:], wT_sb[:], a2_sb[:])
            nc.vector.tensor_copy(wssd_f[:, F:F + 2], wsd_ps[:])
            nc.vector.tensor_copy(wssd[:], wssd_f[:])  # cast to bf16

            t2all_ps = t2p.tile([1, 196], f32, name="t2all")

            def do_group(g):
                """s/d tables + R blocks + totals for blocks 4g..4g+3."""
                sd_src = ltgsd_v[:, 4 * g:4 * g + 4, F:F + 2]
                d_src = ltgsd_v[:, 4 * g:4 * g + 4, F + 1]
                nc.vector.tensor_scalar(negd[:, 4 * g:4 * g + 4], d_src,
                                        -1.0, None, op0=Alu.mult)
                dst = slice(8 * g, 8 * g + 8)
                nc.scalar.activation(uv1[:, dst], sd_src, Act.Exp)
                nc.scalar.activation(uv2[:, dst], sd_src, Act.Exp, scale=0.2)
                if 4 * g + 4 > act_first:
                    # halved tables for sign-assigned blocks in this group
                    nc.scalar.activation(uvh1[:, dst], sd_src, Act.Exp,
                                         bias=nln2_col[:])
                    nc.scalar.activation(uvh2[:, dst], sd_src, Act.Exp,
                                         scale=0.2, bias=nln2_col[:])
                for bb in range(4 * g, 4 * g + 4):
                    ltg_b = ltgsd[:, 66 * bb:66 * bb + F]
                    r0 = RC * bb
                    w1 = uvh1 if bb >= act_first else uv1
                    w2 = uvh2 if bb >= act_first else uv2
                    nc.vector.tensor_scalar(
                        r_all[:, r0:r0 + F], ltg_b,
                        w1[:, 2 * bb + 1:2 * bb + 2], None, op0=Alu.mult)
                    nc.scalar.mul(r_all[:, r0 + F:r0 + 2 * F], ltg_b,
                                  w2[:, 2 * bb + 1:2 * bb + 2])
                # v/v2 den columns, in two sub-slices split at act_first
                g0, g1 = 4 * g, 4 * g + 4
                for lo, hi, wv1, wv2 in (
                        (g0, min(g1, act_first), uv1_v, uv2_v),
                        (max(g0, act_first), g1, uvh1_v, uvh2_v)):
                    if lo >= hi:
                        continue
                    bsl = slice(lo, hi)
                    nc.vector.tensor_copy(r_v[:, bsl, 128], wv1[:, bsl, 1])
                    nc.vector.tensor_copy(r_v[:, bsl, 129], wv2[:, bsl, 1])
                # totals: T2 (cols 0:66) over all blocks (sign blocks hold
                # r/2, so their stationary is a twos vector); TactHalf
                # (cols 66:196) over sign blocks.  Two accumulation regions
                # in one PSUM bank; per-byte zero-on-first-write.
                for bb in range(4 * g, 4 * g + 4):
                    r0 = RC * bb
                    sta = twos_col_bf if bb >= act_first else ones_col_bf
                    nc.tensor.matmul(t2all_ps[0:1, 0:66], sta[:],
                                     r_all[:, r0 + F:r0 + RC],
                                     start=(bb == 0), stop=(bb == NB - 1),
                                     skip_group_check=True)
                    if bb >= act_first:
                        nc.tensor.matmul(t2all_ps[0:1, 66:196],
                                         ones_col_bf[:],
                                         r_all[:, r0:r0 + RC],
                                         start=False, stop=(bb == NB - 1),
                                         skip_group_check=True)

            for g in range(NSUP):
                tp_ps = sps.tile([F_IN, ISUP * P], f32, tag="tp")
                for k in range(ISUP):
                    b = 4 * g + k
                    g_sb = gpool.tile([P, F_IN], f32)
                    nc.sync.dma_start(g_sb[:], graph_d[b * P:(b + 1) * P, :])
                    nc.tensor.transpose(tp_ps[:, k * P:(k + 1) * P], g_sb[:],
                                        identity[:])
                nc.vector.tensor_copy(gT[:, 512 * g:512 * (g + 1)], tp_ps[:])
                pj_ps = sps.tile([P, 66 * ISUP], f32, tag="pj")
                for k in range(ISUP):
                    b = 4 * g + k
                    nc.tensor.matmul(pj_ps[:, 66 * k:66 * (k + 1)],
                                     gT[:, b * P:(b + 1) * P], wssd[:])
                nc.vector.tensor_copy(ltgsd[:, 264 * g:264 * (g + 1)],
                                      pj_ps[:])
                sr_ps = sb2p.tile([2, 512], f32, tag="sr")
                nc.tensor.matmul(sr_ps[:], wssd[:, F:F + 2],
                                 gT[:, 512 * g:512 * (g + 1)])
                nc.scalar.copy(sdrow[:, 512 * g:512 * (g + 1)], sr_ps[:])
                bc_ps = sb2p.tile([P, 512], f32, tag="sr", name="bc_ps")
                nc.tensor.matmul(bc_ps[:], ones_row_bf[:],
                                 sdrow[0:1, 512 * g:512 * (g + 1)])
                nc.scalar.copy(s_rep[:, 512 * g:512 * (g + 1)], bc_ps[:])
                do_group(g)
                for bb in range(4 * g, 4 * g + 4):
                    emit_mask(0, bb)

            # broadcast totals down partitions: [1,196] -> [128,196]
            t2all_sb = ssb.tile([1, 196], f32, tag="t2sb")
            nc.vector.tensor_copy(t2all_sb[:], t2all_ps[:])
            t2rep_ps = sps.tile([P, 196], f32, tag="tp")
            nc.tensor.matmul(t2rep_ps[:], ones_row[:], t2all_sb[:])
            nc.scalar.copy(t2rep[:], t2rep_ps[:])
            for t in range(ISUP):
                nc.vector.tensor_copy(tact4[:, RC * t:RC * (t + 1)],
                                      t2rep[:, 66:196])

        # ---- main masked-matmul loop ----
        accp = ctx.enter_context(
            tc.tile_pool(name="acc", bufs=2, space="PSUM"))

        def late_phase(sup):
            """Epilogue on SBUF snapshots eps == masked sums (A)."""
            i0 = sup * ISUP
            e_base = (sup % 2) * RC * ISUP
            es = eps_all[:, e_base:e_base + RC * ISUP]
            es_v = es.rearrange("p (t c) -> p t c", c=RC)
            u_v = uv1_v[:, i0:i0 + ISUP, 0]
            u2_v = uv2_v[:, i0:i0 + ISUP, 0]
            den1 = ep.tile([P, ISUP], f32, tag="den1", name="den1")
            nc.vector.tensor_tensor(den1[:], u_v, es_v[:, :, 128],
                                    op=Alu.mult)
            dd = ep.tile([P, ISUP], f32, tag="dd", name="dd")
            nc.vector.tensor_tensor(dd[:],
                                    t2rep[:, 65:66].to_broadcast([P, ISUP]),
                                    es_v[:, :, 129], op=Alu.subtract)
            den2 = ep.tile([P, ISUP], f32, tag="den2", name="den2")
            nc.vector.tensor_tensor(den2[:], dd[:], u2_v, op=Alu.mult)
            den = ep.tile([P, ISUP], f32, tag="den", name="den")
            nc.vector.tensor_tensor(den[:], den2[:], den1[:], op=Alu.add)
            rden = ep.tile([P, ISUP], f32, tag="rden", name="rden")
            nc.vector.reciprocal(rden[:], den[:])
            for t in range(ISUP):
                i = i0 + t
                e0 = e_base + RC * t
                u_b = uv1[:, 2 * i:2 * i + 1].to_broadcast([P, F])
                u2_b = uv2[:, 2 * i:2 * i + 1].to_broadcast([P, F])
                n1 = ep.tile([P, F], f32, tag="n1", name="n1")
                nc.vector.tensor_tensor(n1[:], eps_all[:, e0:e0 + F], u_b,
                                        op=Alu.mult)
                d2 = ep.tile([P, F], f32, tag="d2", name="d2")
                nc.vector.tensor_tensor(d2[:], t2rep[:, 0:F],
                                        eps_all[:, e0 + F:e0 + 2 * F],
                                        op=Alu.subtract)
                n2 = ep.tile([P, F], f32, tag="n2", name="n2")
                nc.vector.tensor_tensor(n2[:], d2[:], u2_b, op=Alu.mult)
                num = ep.tile([P, F], f32, tag="num", name="num")
                nc.vector.tensor_tensor(num[:], n1[:], n2[:], op=Alu.add)
                ot = ep.tile([P, F], f32, tag="ot", name="ot")
                nc.vector.tensor_tensor(
                    ot[:], num[:], rden[:, t:t + 1].to_broadcast([P, F]),
                    op=Alu.mult)
                nc.sync.dma_start(out_d[i * P:(i + 1) * P, :], ot[:])

        for sup in range(NSUP):
            if sup + 1 < NSUP:
                for b in range(NB):
                    emit_mask(sup + 1, b)
            mtiles = [mask_tiles.pop((sup, b)) for b in range(NB)]

            acc = accp.tile([P, 512 * ISUP], f32, tag="acc", name=f"acc{sup}")
            for b in range(NB):
                r0 = RC * b
                for t in range(ISUP):
                    nc.tensor.matmul(
                        acc[:, 512 * t:512 * t + RC],
                        mtiles[b][:, t * P:(t + 1) * P],
                        r_all[:, r0:r0 + RC],
                        start=(b == 0), stop=(b == NB - 1))
            # batched snapshot psum -> SBUF with +TactHalf folded in
            e_base = (sup % 2) * RC * ISUP
            eps3 = eps_all[:, e_base:e_base + RC * ISUP].rearrange(
                "p (t c) -> p t c", c=RC)
            acc3 = acc.rearrange("p (t c) -> p t c", c=512)
            nc.vector.tensor_tensor(eps3, acc3[:, :, 0:RC],
                                    tact4[:].rearrange(
                                        "p (t c) -> p t c", c=RC),
                                    op=Alu.add)
            if sup > 0:
                late_phase(sup - 1)
        late_phase(NSUP - 1)

    nc.compile()
    return nc


def _get_nc():
    if "nc" not in _CACHE:
        _CACHE["nc"] = _build()
    return _CACHE["nc"]


def kernel(graph, W, a):
    from concourse.bass_utils import run_bass_kernel_spmd

    graph = np.ascontiguousarray(np.asarray(graph, dtype=np.float32))
    W = np.asarray(W, dtype=np.float32)
    a = np.asarray(a, dtype=np.float32)

    nc = _get_nc()
    in_maps = [
        {
            "graph": graph,
            "w": np.ascontiguousarray(W[h]),
            "a": np.ascontiguousarray(a[h].reshape(2, F)),
        }
        for h in range(H)
    ]
    trace = bool(int(os.environ.get("GAT_TRACE", "0")))
    res = run_bass_kernel_spmd(nc, in_maps, core_ids=list(range(H)), trace=trace)
    _CACHE["last_result"] = res
    return np.stack([res.results[h]["out"] for h in range(H)], axis=0)


# revision 18
# speedup vs baseline: 5.5014x; 1.0468x over previous
"""Multi-head GAT layer for Trainium2 — 8 heads sharded across 8 NeuronCores.

Per head h (N=4096 nodes, F=64 features):
    ltg   = graph @ W[h]                          [N, F]
    s     = ltg @ a_src,  d = ltg @ a_dst         [N]
    E     = leaky_relu(s[:, None] + d[None, :], 0.2)
    Alpha = softmax(E, axis=-1)
    out   = Alpha @ ltg

Algebraic trick (exact): with z = s_i + d_j and M_ij = [z >= 0],

    exp(leaky_relu(z)) = M_ij e^{s_i} e^{d_j} + (1-M_ij) e^{.2 s_i} e^{.2 d_j}

so the N x N softmax reduces to one mask-materialization pass plus masked
matmuls on the PE against R = [v .* ltg | v2 .* ltg | v | v2] (130 bf16
columns per j-block), with the (1-M) path recovered as total-minus-masked.

Mask generation is split across DVE and the Activation engine with
consistent algebra:
  - DVE blocks emit m = [z >= 0] in {0, 1} (single-op tensor_scalar is_ge;
    two-op forms and GpSimd fall off the HW fast path by 10-30x).
  - Act blocks emit m = sign(z) in {-1, +1} (Sign shares the Exp activation
    table — no 1.3us table reload), and use HALVED R columns
    (vh = e^{d - ln2} etc.), so their matmul contribution is
    sign @ (r/2) = A_b - T_b/2.
Summing over j-blocks:  P = A_total - TactHalf, with TactHalf the column
totals of the halved R over sign-assigned blocks.  The +TactHalf correction
is folded into the PSUM->SBUF snapshot, after which eps == A_total exactly
and the plain complement algebra applies.  Sign blocks contribute their
UNHALVED totals to T2 via a twos-valued stationary vector.

vs the v1 kernel: bf16 projection/setup matmuls (f32 matmul runs at 4
cycles/row), batched setup copies, totals accumulated on the fly, PSUM
accumulator double-buffered across supersteps, masks prefetched one
superstep ahead, and the whole epilogue on DVE with broadcast multipliers.

Heads are fully independent: core h computes head h; no collectives.
"""

import math
import os
from contextlib import ExitStack

import numpy as np

N, F_IN, F, H = 4096, 64, 64, 8
P = 128
NB = N // P           # 32 node blocks
ISUP = 4              # i-blocks per PSUM super-block (4 banks of accumulators)
NSUP = NB // ISUP     # 8 super iterations
RC = 130              # R columns per j-block: [v.ltg | v2.ltg | v | v2]

# j-block -> mask engine split: blocks [0, NB-NB_ACT) on DVE (is_ge {0,1}),
# blocks [NB-NB_ACT, NB) on Act (sign {-1,+1} with halved R)
NB_ACT = int(os.environ.get("GAT_NB_ACT", "13"))
_CACHE = {}


def _build():
    import concourse.bass as bass  # noqa: F401
    import concourse.mybir as mybir
    import concourse.tile as tile
    from concourse import bacc

    dt = mybir.dt
    f32 = dt.float32
    bf16 = dt.bfloat16
    Alu = mybir.AluOpType
    Act = mybir.ActivationFunctionType

    act_first = NB - NB_ACT
    LN2 = float(math.log(2.0))

    nc = bacc.Bacc("TRN2", debug=False, num_devices=H)
    graph_d = nc.dram_tensor("graph", [N, F_IN], f32, kind="ExternalInput").ap()
    w_d = nc.dram_tensor("w", [F_IN, F], f32, kind="ExternalInput").ap()
    a_d = nc.dram_tensor("a", [2, F], f32, kind="ExternalInput").ap()
    out_d = nc.dram_tensor("out", [N, F], f32, kind="ExternalOutput").ap()

    ident_d = nc.inline_tensor(np.eye(P, dtype=np.float32), name="ident")

    with tile.TileContext(nc) as tc, ExitStack() as ctx:
        persist = ctx.enter_context(tc.tile_pool(name="persist", bufs=1))
        gpool = ctx.enter_context(tc.tile_pool(name="gp", bufs=32))
        mp = ctx.enter_context(tc.tile_pool(name="mask", bufs=2))
        ep = ctx.enter_context(tc.tile_pool(name="ep", bufs=6))
        ssb = ctx.enter_context(tc.tile_pool(name="ssb", bufs=2))

        identity = persist.tile([P, P], f32)
        nc.sync.dma_start(identity[:], ident_d.ap())
        ones_col_bf = persist.tile([P, 1], bf16)
        nc.vector.memset(ones_col_bf[:], 1.0)
        twos_col_bf = persist.tile([P, 1], bf16)
        nc.vector.memset(twos_col_bf[:], 2.0)
        ones_row = persist.tile([1, P], f32)
        nc.vector.memset(ones_row[:], 1.0)
        ones_row_bf = persist.tile([1, P], bf16)
        nc.vector.memset(ones_row_bf[:], 1.0)
        nln2_col = persist.tile([P, 1], f32)
        nc.vector.memset(nln2_col[:], -LN2)

        wssd_f = persist.tile([F_IN, F + 2], f32)   # staging: [W | w_s | w_d]
        nc.sync.dma_start(wssd_f[:, 0:F], w_d[:])
        a2_sb = persist.tile([F, 2], f32)
        nc.sync.dma_start(a2_sb[:], a_d.rearrange("t k -> k t"))
        wssd = persist.tile([F_IN, F + 2], bf16)    # bf16 working copy

        gT = persist.tile([F_IN, N], bf16)           # graph^T (bf16)
        ltgsd = persist.tile([P, 66 * NB], f32)      # per b: ltg (64) | s | d
        negd = persist.tile([P, NB], f32)            # -d columns (is_ge bias)
        uv1 = persist.tile([P, 2 * NB], f32)         # exp(s), exp(d)
        uv2 = persist.tile([P, 2 * NB], f32)         # exp(.2 s), exp(.2 d)
        uvh1 = persist.tile([P, 2 * NB], f32)        # exp(. - ln2) (act blocks)
        uvh2 = persist.tile([P, 2 * NB], f32)
        sdrow = persist.tile([2, N], bf16)           # s, d rows
        s_rep = persist.tile([P, N], bf16)           # s broadcast down parts
        r_all = persist.tile([P, RC * NB], bf16)     # [R1|R2|v|v2] per b
        t2rep = persist.tile([P, 196], f32)          # [T2(66)|TactHalf(130)]
        tact4 = persist.tile([P, RC * ISUP], f32)    # TactHalf repeated 4x
        t64rep4 = persist.tile([P, F * ISUP], f32)   # T2[0:64] repeated 4x
        eps_all = persist.tile([P, RC * ISUP * 2], f32)  # snapshots ping-pong

        ltgsd_v = ltgsd.rearrange("p (b c) -> p b c", c=66)
        r_v = r_all.rearrange("p (b c) -> p b c", c=RC)
        uv1_v = uv1.rearrange("p (b c) -> p b c", c=2)
        uv2_v = uv2.rearrange("p (b c) -> p b c", c=2)
        uvh1_v = uvh1.rearrange("p (b c) -> p b c", c=2)
        uvh2_v = uvh2.rearrange("p (b c) -> p b c", c=2)

        mask_tiles = {}

        def emit_mask(sup, b):
            i0 = sup * ISUP * P
            srep = s_rep[:, i0:i0 + ISUP * P]
            mt = mp.tile([P, ISUP * P], bf16, tag=f"m{b}", name=f"mask{b}")
            if b >= act_first:
                # sign(s_i + d_j) in {-1,+1}; Sign shares the Exp act table
                nc.scalar.sign(mt[:], srep,
                               bias=ltgsd[:, 66 * b + 65:66 * b + 66])
            else:
                nc.vector.tensor_scalar(mt[:], srep, negd[:, b:b + 1], None,
                                        op0=Alu.is_ge)
            mask_tiles[(sup, b)] = mt

        # ---- setup: projection, s/d, R tables, totals ----
        with ExitStack() as sctx:
            sps = sctx.enter_context(
                tc.tile_pool(name="sps", bufs=2, space="PSUM"))
            sb2p = sctx.enter_context(
                tc.tile_pool(name="sb2", bufs=2, space="PSUM"))
            t2p = sctx.enter_context(
                tc.tile_pool(name="t2p", bufs=1, space="PSUM"))

            # W^T, then [w_s | w_d] = (W^T).T @ a2 ; all-f32, one-off
            wT_ps = sps.tile([F, F_IN], f32, tag="tp")
            nc.tensor.transpose(wT_ps[:], wssd_f[:, 0:F],
                                identity[0:F_IN, 0:F_IN])
            wT_sb = ssb.tile([F, F_IN], f32)
            nc.vector.tensor_copy(wT_sb[:], wT_ps[:])
            wsd_ps = sps.tile([F_IN, 2], f32, tag="pj")
            nc.tensor.matmul(wsd_ps[:], wT_sb[:], a2_sb[:])
            nc.vector.tensor_copy(wssd_f[:, F:F + 2], wsd_ps[:])
            nc.vector.tensor_copy(wssd[:], wssd_f[:])  # cast to bf16

            t2all_ps = t2p.tile([1, 196], f32, name="t2all")

            def do_uv(gp2):
                """exp tables for group pair (2*gp2, 2*gp2+1), batched."""
                sd_src = ltgsd_v[:, 8 * gp2:8 * gp2 + 8, F:F + 2]
                dst = slice(16 * gp2, 16 * gp2 + 16)
                nc.scalar.activation(uv1[:, dst], sd_src, Act.Exp)
                nc.scalar.activation(uv2[:, dst], sd_src, Act.Exp, scale=0.2)
                if 8 * gp2 + 8 > act_first:
                    # halved tables for sign-assigned blocks in this pair
                    nc.scalar.activation(uvh1[:, dst], sd_src, Act.Exp,
                                         bias=nln2_col[:])
                    nc.scalar.activation(uvh2[:, dst], sd_src, Act.Exp,
                                         scale=0.2, bias=nln2_col[:])

            def do_r(g):
                """R blocks + totals for blocks 4g..4g+3 (needs uv)."""
                g0, g1 = 4 * g, 4 * g + 4
                for lo, hi, wv1, wv2 in (
                        (g0, min(g1, act_first), uv1_v, uv2_v),
                        (max(g0, act_first), g1, uvh1_v, uvh2_v)):
                    if lo >= hi:
                        continue
                    bsl = slice(lo, hi)
                    nw = hi - lo
                    ltg_s = ltgsd_v[:, bsl, 0:F]
                    nc.vector.tensor_tensor(
                        r_v[:, bsl, 0:F], ltg_s,
                        wv1[:, bsl, 1:2].to_broadcast([P, nw, F]),
                        op=Alu.mult)
                    nc.vector.tensor_tensor(
                        r_v[:, bsl, F:2 * F], ltg_s,
                        wv2[:, bsl, 1:2].to_broadcast([P, nw, F]),
                        op=Alu.mult)
                    nc.vector.tensor_copy(r_v[:, bsl, 128], wv1[:, bsl, 1])
                    nc.vector.tensor_copy(r_v[:, bsl, 129], wv2[:, bsl, 1])
                # totals: T2 (cols 0:66) over all blocks (sign blocks hold
                # r/2, so their stationary is a twos vector); TactHalf
                # (cols 66:196) over sign blocks.  Two accumulation regions
                # in one PSUM bank; per-byte zero-on-first-write.
                for bb in range(4 * g, 4 * g + 4):
                    r0 = RC * bb
                    sta = twos_col_bf if bb >= act_first else ones_col_bf
                    nc.tensor.matmul(t2all_ps[0:1, 0:66], sta[:],
                                     r_all[:, r0 + F:r0 + RC],
                                     start=(bb == 0), stop=(bb == NB - 1),
                                     skip_group_check=True)
                    if bb >= act_first:
                        nc.tensor.matmul(t2all_ps[0:1, 66:196],
                                         ones_col_bf[:],
                                         r_all[:, r0:r0 + RC],
                                         start=False, stop=(bb == NB - 1),
                                         skip_group_check=True)

            g_sbs = []
            for b in range(NB):
                g_sb = gpool.tile([P, F_IN], f32)
                nc.sync.dma_start(g_sb[:], graph_d[b * P:(b + 1) * P, :])
                g_sbs.append(g_sb)

            for g in range(NSUP):
                tp_ps = sps.tile([F_IN, ISUP * P], f32, tag="tp")
                for k in range(ISUP):
                    nc.tensor.transpose(tp_ps[:, k * P:(k + 1) * P],
                                        g_sbs[4 * g + k][:], identity[:])
                nc.vector.tensor_copy(gT[:, 512 * g:512 * (g + 1)], tp_ps[:])
                pj_ps = sps.tile([P, 66 * ISUP], f32, tag="pj")
                for k in range(ISUP):
                    b = 4 * g + k
                    nc.tensor.matmul(pj_ps[:, 66 * k:66 * (k + 1)],
                                     gT[:, b * P:(b + 1) * P], wssd[:])
                nc.vector.tensor_copy(ltgsd[:, 264 * g:264 * (g + 1)],
                                      pj_ps[:])
                sr_ps = sb2p.tile([2, 512], f32, tag="sr")
                nc.tensor.matmul(sr_ps[:], wssd[:, F:F + 2],
                                 gT[:, 512 * g:512 * (g + 1)])
                nc.scalar.copy(sdrow[:, 512 * g:512 * (g + 1)], sr_ps[:])
                bc_ps = sb2p.tile([P, 512], f32, tag="sr", name="bc_ps")
                nc.tensor.matmul(bc_ps[:], ones_row_bf[:],
                                 sdrow[0:1, 512 * g:512 * (g + 1)])
                nc.scalar.copy(s_rep[:, 512 * g:512 * (g + 1)], bc_ps[:])
                d_src = ltgsd_v[:, 4 * g:4 * g + 4, F + 1]
                nc.vector.tensor_scalar(negd[:, 4 * g:4 * g + 4], d_src,
                                        -1.0, None, op0=Alu.mult)
                for bb in range(4 * g, 4 * g + 4):
                    emit_mask(0, bb)
                if g % 2 == 1:
                    do_uv(g // 2)
                    do_r(g - 1)
                    do_r(g)

            # broadcast totals down partitions: [1,196] -> [128,196]
            t2all_sb = ssb.tile([1, 196], f32, tag="t2sb")
            nc.vector.tensor_copy(t2all_sb[:], t2all_ps[:])
            t2rep_ps = sps.tile([P, 196], f32, tag="tp")
            nc.tensor.matmul(t2rep_ps[:], ones_row[:], t2all_sb[:])
            nc.scalar.copy(t2rep[:], t2rep_ps[:])
            for t in range(ISUP):
                nc.vector.tensor_copy(tact4[:, RC * t:RC * (t + 1)],
                                      t2rep[:, 66:196])
                nc.vector.tensor_copy(t64rep4[:, F * t:F * (t + 1)],
                                      t2rep[:, 0:F])

        # ---- main masked-matmul loop ----
        accp = ctx.enter_context(
            tc.tile_pool(name="acc", bufs=2, space="PSUM"))

        def late_phase(sup):
            """Epilogue on SBUF snapshots eps == masked sums (A)."""
            i0 = sup * ISUP
            e_base = (sup % 2) * RC * ISUP
            es = eps_all[:, e_base:e_base + RC * ISUP]
            es_v = es.rearrange("p (t c) -> p t c", c=RC)
            u_v = uv1_v[:, i0:i0 + ISUP, 0]
            u2_v = uv2_v[:, i0:i0 + ISUP, 0]
            den1 = ep.tile([P, ISUP], f32, tag="den1", name="den1")
            nc.vector.tensor_tensor(den1[:], u_v, es_v[:, :, 128],
                                    op=Alu.mult)
            dd = ep.tile([P, ISUP], f32, tag="dd", name="dd")
            nc.vector.tensor_tensor(dd[:],
                                    t2rep[:, 65:66].to_broadcast([P, ISUP]),
                                    es_v[:, :, 129], op=Alu.subtract)
            den2 = ep.tile([P, ISUP], f32, tag="den2", name="den2")
            nc.vector.tensor_tensor(den2[:], dd[:], u2_v, op=Alu.mult)
            den = ep.tile([P, ISUP], f32, tag="den", name="den")
            nc.vector.tensor_tensor(den[:], den2[:], den1[:], op=Alu.add)
            rden = ep.tile([P, ISUP], f32, tag="rden", name="rden")
            nc.vector.reciprocal(rden[:], den[:])
            # batched [P, ISUP*F] epilogue with per-i-block broadcast scalars
            es1 = es_v[:, :, 0:F]
            es2 = es_v[:, :, F:2 * F]
            u_b = uv1_v[:, i0:i0 + ISUP, 0:1].to_broadcast([P, ISUP, F])
            u2_b = uv2_v[:, i0:i0 + ISUP, 0:1].to_broadcast([P, ISUP, F])
            n1 = ep.tile([P, ISUP * F], f32, tag="n1", name="n1")
            nc.vector.tensor_tensor(
                n1.rearrange("p (t c) -> p t c", c=F), es1, u_b, op=Alu.mult)
            d2 = ep.tile([P, ISUP * F], f32, tag="d2", name="d2")
            nc.vector.tensor_tensor(
                d2.rearrange("p (t c) -> p t c", c=F),
                t64rep4[:].rearrange("p (t c) -> p t c", c=F), es2,
                op=Alu.subtract)
            n2 = ep.tile([P, ISUP * F], f32, tag="n2", name="n2")
            nc.vector.tensor_tensor(
                n2.rearrange("p (t c) -> p t c", c=F),
                d2.rearrange("p (t c) -> p t c", c=F), u2_b, op=Alu.mult)
            num = ep.tile([P, ISUP * F], f32, tag="num", name="num")
            nc.vector.tensor_tensor(num[:], n1[:], n2[:], op=Alu.add)
            ot = ep.tile([P, ISUP * F], f32, tag="ot", name="ot")
            nc.vector.tensor_tensor(
                ot.rearrange("p (t c) -> p t c", c=F),
                num.rearrange("p (t c) -> p t c", c=F),
                rden[:].rearrange("p (t o) -> p t o", o=1).to_broadcast(
                    [P, ISUP, F]),
                op=Alu.mult)
            for t in range(ISUP):
                i = i0 + t
                nc.sync.dma_start(out_d[i * P:(i + 1) * P, :],
                                  ot[:, F * t:F * (t + 1)])

        for sup in range(NSUP):
            if sup + 1 < NSUP:
                for b in range(NB):
                    emit_mask(sup + 1, b)
            mtiles = [mask_tiles.pop((sup, b)) for b in range(NB)]

            acc = accp.tile([P, 512 * ISUP], f32, tag="acc", name=f"acc{sup}")
            for b in range(NB):
                r0 = RC * b
                for t in range(ISUP):
                    nc.tensor.matmul(
                        acc[:, 512 * t:512 * t + RC],
                        mtiles[b][:, t * P:(t + 1) * P],
                        r_all[:, r0:r0 + RC],
                        start=(b == 0), stop=(b == NB - 1))
            # batched snapshot psum -> SBUF with +TactHalf folded in
            e_base = (sup % 2) * RC * ISUP
            eps3 = eps_all[:, e_base:e_base + RC * ISUP].rearrange(
                "p (t c) -> p t c", c=RC)
            acc3 = acc.rearrange("p (t c) -> p t c", c=512)
            nc.vector.tensor_tensor(eps3, acc3[:, :, 0:RC],
                                    tact4[:].rearrange(
                                        "p (t c) -> p t c", c=RC),
                                    op=Alu.add)
            if sup > 0:
                late_phase(sup - 1)
        late_phase(NSUP - 1)

    nc.compile()
    return nc


def _get_nc():
    if "nc" not in _CACHE:
        _CACHE["nc"] = _build()
    return _CACHE["nc"]


def kernel(graph, W, a):
    from concourse.bass_utils import run_bass_kernel_spmd

    graph = np.ascontiguousarray(np.asarray(graph, dtype=np.float32))
    W = np.asarray(W, dtype=np.float32)
    a = np.asarray(a, dtype=np.float32)

    nc = _get_nc()
    in_maps = [
        {
            "graph": graph,
            "w": np.ascontiguousarray(W[h]),
            "a": np.ascontiguousarray(a[h].reshape(2, F)),
        }
        for h in range(H)
    ]
    trace = bool(int(os.environ.get("GAT_TRACE", "0")))
    res = run_bass_kernel_spmd(nc, in_maps, core_ids=list(range(H)), trace=trace)
    _CACHE["last_result"] = res
    return np.stack([res.results[h]["out"] for h in range(H)], axis=0)
